# revision 17
# baseline (speedup 1.0000x reference)
"""Trainium2 Bass kernel for nn_LoRALinear (DoRA-style LoRA linear).

Reference math (per problem):
    base = x @ W^T
    lora = sc * (x @ A^T) @ B^T          (sc = 2.0)
    w_eff = W + sc * (B @ A)
    s = magnitude / ||w_eff||_row         (row norm over in_dim)
    out = base + (s - 1) * base + s * lora = s * (base + lora)
        = x @ (s[:, None] * w_eff)^T

So the whole op collapses to one dense matmul with a derived weight.

Strategy: data-parallel shard x over batch*seq across 8 cores; every core
redundantly derives w_eff (bf16) + row norms + scale s on device from the
small replicated weights, then computes its x-shard's matmul in bf16
(fp32 accumulate) on the PE array.

The host pre-stages layout only: x is transposed to d-major tiles and
rounded to bf16 (as are W^T / A / B^T), so the device spends zero PE
cycles transposing and streams the matmul at 1 column/cycle. All
arithmetic (w_eff derivation, norms, rsqrt, scaling, the big matmul)
runs on device.

Schedule notes (from trace analysis):
  - dummy warm-up matmuls ramp the PE p-state while the first DMAs land
  - x chunks stream on the gpsimd DGE queue so out-DMA semaphore waits
    on the sync queue can never delay an x prefetch
  - w_eff adds split across DVE (h=0) and gpsimd (h=1) so the setup
    pace matches the PE; BAT psum has its own 2-buf tag so the main
    6-buf rotation never couples to setup latencies
  - the first two token tiles interleave with the 8-step w_eff loop
    k-by-k; norm matmuls run after them, so s (needed only by output
    scaling) is off the PE critical path
  - PSUM drains are plain ACT copies (no s dependency) so psum recycles
    immediately; the s scale is applied by DVE in SBUF afterwards
"""

import os
import numpy as np
from contextlib import ExitStack

import ml_dtypes

import concourse.bass as bass
import concourse.mybir as mybir
import concourse.tile as tile
from concourse import bacc
from concourse.bass import ts
from concourse.bass_utils import run_bass_kernel_spmd

N_CORES = 8
B, S, D_IN, D_OUT, R = 4, 8192, 1024, 1024, 16
SCALING = 32.0 / 16.0
M_TOT = B * S                 # 32768 tokens
M_CORE = M_TOT // N_CORES     # 4096 tokens per core
P = 128
CHUNK_T = 512                 # tokens per DMA chunk
N_CHUNKS = M_CORE // CHUNK_T  # 8
TPC = CHUNK_T // P            # 4 t-tiles per chunk
K_TILES = D_IN // P           # 8
NH = D_OUT // 512             # 2 n-halves of 512
N_WARM = 13                   # PE p-state warm-up matmuls
F32 = mybir.dt.float32
BF16 = mybir.dt.bfloat16
NPBF16 = ml_dtypes.bfloat16


def _kernel_body(ctx: ExitStack, tc: "tile.TileContext", xp, wT, abp, mag, out):
    nc = tc.nc
    const_pool = ctx.enter_context(tc.tile_pool(name="const", bufs=1))
    wt_pool = ctx.enter_context(tc.tile_pool(name="wt", bufs=8))
    w_pool = ctx.enter_context(tc.tile_pool(name="w", bufs=1))
    sq_pool = ctx.enter_context(tc.tile_pool(name="sq", bufs=8))
    x_pool = ctx.enter_context(tc.tile_pool(name="x", bufs=4))
    o_pool = ctx.enter_context(tc.tile_pool(name="o", bufs=8))
    ps_pool = ctx.enter_context(tc.tile_pool(name="ps", bufs=4, space="PSUM"))
    ps_bat = ctx.enter_context(tc.tile_pool(name="ps_bat", bufs=4, space="PSUM"))
    dram_pool = ctx.enter_context(tc.tile_pool(name="dram", bufs=1, space="DRAM"))

    # ---- PE warm-up: dependency-free matmuls ramp the clock while the
    # first DMAs are still in flight ----
    warm_in = const_pool.tile([P, 512], BF16)
    nc.vector.memset(warm_in[:], 0.0)
    for i in range(N_WARM):
        wps = ps_pool.tile([P, 512], F32, tag="mm", name=f"warm{i}")
        nc.tensor.matmul(
            wps[:], lhsT=warm_in[:, :P], rhs=warm_in[:], start=True, stop=True
        )

    # ---- x chunk prefetch on the gpsimd DGE queue; the queue self-paces
    # on the 4-buf rotation, so all 8 issues go in upfront. Chunk 0 loads
    # in two halves so the first k-tiles land sooner. ----
    xch = []
    HK = K_TILES * CHUNK_T // 2

    def issue_x(c):
        t_ = x_pool.tile([P, K_TILES * CHUNK_T], BF16, tag="x", name=f"x{c}")
        if c == 0:
            nc.gpsimd.dma_start(out=t_[:, :HK], in_=xp[ts(c, P), :HK])
            nc.gpsimd.dma_start(out=t_[:, HK:], in_=xp[ts(c, P), HK:])
        else:
            nc.gpsimd.dma_start(out=t_[:], in_=xp[ts(c, P), :])
        xch.append(t_)

    for c in range(4):
        issue_x(c)

    # ---- constants / small inputs ----
    ab_sb = const_pool.tile([R, 2 * D_IN], BF16)
    nc.sync.dma_start(ab_sb[:], abp[:, :])
    a2v = ab_sb[:, :D_IN]
    bTv = ab_sb[:, D_IN:]
    mag_sb = const_pool.tile([1, D_OUT], F32)
    nc.scalar.dma_start(mag_sb[:], mag[:, :])
    ones_f = const_pool.tile([P, 1], F32)
    nc.vector.memset(ones_f[:], 1.0)
    ones = const_pool.tile([P, 1], BF16)
    nc.vector.tensor_copy(ones[:], ones_f[:])
    # prewarm the Ln/Exp ACT tables during the idle head so the s-chain
    # doesn't pay the 1.3us table loads mid-kernel
    tbl = const_pool.tile([1, 1], F32)
    nc.scalar.activation(tbl[:], ones_f[:1, :1], mybir.ActivationFunctionType.Ln)
    nc.scalar.activation(
        tbl[:], ones_f[:1, :1], mybir.ActivationFunctionType.Exp, bias=0.0, scale=-0.5
    )

    # ---- w_eff^T derivation with the first two token tiles interleaved
    # k-by-k so the PE never waits on the weight DMA stream ----
    t0_ps = [
        ps_pool.tile([P, 512], F32, tag="mm", name=f"pst0_{h}") for h in range(NH)
    ]
    weff = []
    sqs = []
    for k in range(K_TILES):
        wt = wt_pool.tile([P, D_OUT], BF16, tag="wt", name=f"wt{k}")
        nc.sync.dma_start(wt[:], wT[ts(k, P), :])
        weff_k = w_pool.tile([P, D_OUT], BF16, tag=f"weff{k}", name=f"weff{k}")
        for h in range(NH):
            bat = ps_bat.tile([P, 512], F32, tag="bat", name=f"bat{k}_{h}")
            nc.tensor.matmul(
                bat[:],
                lhsT=a2v[:, ts(k, P)],
                rhs=bTv[:, ts(h, 512)],
                start=True,
                stop=True,
            )
            # fp32 add on DVE, rounded to bf16 on write
            nc.vector.tensor_add(weff_k[:, ts(h, 512)], wt[:, ts(h, 512)], bat[:])
        sqt = sq_pool.tile([P, D_OUT], BF16, tag="sq", name=f"sq{k}")
        nc.scalar.square(sqt[:], weff_k[:])
        sqs.append(sqt)
        for h in range(NH):
            nc.tensor.matmul(
                t0_ps[h][:],
                lhsT=xch[0][:, k * CHUNK_T : k * CHUNK_T + P],
                rhs=weff_k[:, ts(h, 512)],
                start=(k == 0),
                stop=(k == K_TILES - 1),
            )
        weff.append(weff_k)

    # chunk-0 token tiles: matmul + ACT drain now; the s scale and the
    # store are deferred until s_rep exists (DVE s-chain ops must precede
    # any drain multiply in DVE program order)
    deferred = []
    o_sb = o_pool.tile([P, D_OUT], F32, tag="o", name="o_t0")
    for h in range(NH):
        nc.scalar.copy(o_sb[:, ts(h, 512)], t0_ps[h][:])
    deferred.append((0, o_sb))
    for mt in range(1, TPC):
        pss = [
            ps_pool.tile([P, 512], F32, tag="mm", name=f"pso0_{mt}_{h}")
            for h in range(NH)
        ]
        for k in range(K_TILES):
            lhsT = xch[0][:, k * CHUNK_T + mt * P : k * CHUNK_T + (mt + 1) * P]
            for h in range(NH):
                nc.tensor.matmul(
                    pss[h][:],
                    lhsT=lhsT,
                    rhs=weff[k][:, ts(h, 512)],
                    start=(k == 0),
                    stop=(k == K_TILES - 1),
                )
        o_sb = o_pool.tile([P, D_OUT], F32, tag="o", name=f"o0_{mt}")
        for h in range(NH):
            nc.scalar.copy(o_sb[:, ts(h, 512)], pss[h][:])
        deferred.append((mt, o_sb))

    # ---- row-norm^2 via ones-matmul over squared tiles (PE work placed
    # after chunk 0 so sq_7 is long since ready) ----
    norm2_ps = [
        ps_bat.tile([1, 512], F32, tag="bat", name=f"norm2_{h}") for h in range(NH)
    ]
    for k in range(K_TILES):
        for h in range(NH):
            nc.tensor.matmul(
                norm2_ps[h][:],
                lhsT=ones[:],
                rhs=sqs[k][:, ts(h, 512)],
                start=(k == 0),
                stop=(k == K_TILES - 1),
            )

    # ---- s = mag / sqrt(norm2), refined; broadcast to all partitions ----
    norm2_sb = const_pool.tile([1, D_OUT], F32)
    for h in range(NH):
        nc.scalar.copy(norm2_sb[:, ts(h, 512)], norm2_ps[h][:])
    # rsqrt(n) = exp(-0.5 * ln(n)), then one Newton step to kill LUT error
    lnn = const_pool.tile([1, D_OUT], F32)
    nc.scalar.activation(lnn[:], norm2_sb[:], mybir.ActivationFunctionType.Ln)
    y = const_pool.tile([1, D_OUT], F32)
    nc.scalar.activation(
        y[:], lnn[:], mybir.ActivationFunctionType.Exp, bias=0.0, scale=-0.5
    )
    t = const_pool.tile([1, D_OUT], F32)
    nc.vector.tensor_mul(t[:], y[:], y[:])     # Newton: y <- y*(1.5 - 0.5*n*y^2)
    nc.vector.tensor_mul(t[:], t[:], norm2_sb[:])
    nc.vector.tensor_scalar(
        t[:], t[:], -0.5, 1.5, mybir.AluOpType.mult, mybir.AluOpType.add
    )
    nc.vector.tensor_mul(y[:], y[:], t[:])
    s1 = const_pool.tile([1, D_OUT], F32)
    nc.vector.tensor_mul(s1[:], mag_sb[:], y[:])
    # broadcast s to all 128 partitions via a DRAM round trip with a
    # stride-0 partition read
    s_dram = dram_pool.tile([1, D_OUT], F32)
    nc.sync.dma_start(s_dram[:], s1[:])
    sd = s_dram[:]
    s_bcast_ap = bass.AP(tensor=sd.tensor, offset=sd.offset, ap=[[0, P], *sd.ap])
    s_rep = const_pool.tile([P, D_OUT], F32)
    nc.sync.dma_start(out=s_rep[:], in_=s_bcast_ap)

    # deferred chunk-0 scale + store
    for mt, o_sb in deferred:
        nc.vector.tensor_mul(
            o_sb[:, ts(0, 512)], o_sb[:, ts(0, 512)], s_rep[:, ts(0, 512)]
        )
        nc.gpsimd.tensor_mul(
            o_sb[:, ts(1, 512)], o_sb[:, ts(1, 512)], s_rep[:, ts(1, 512)]
        )
        nc.sync.dma_start(out[ts(mt, P), :], o_sb[:])

    # ---- main loop over 512-token chunks ----
    # xp rows c*128+p hold x^T data: xp[c*128+p, k*512+t] = x[c*512+t, k*128+p]
    for c in range(1, N_CHUNKS):
        if c + 3 < N_CHUNKS:
            issue_x(c + 3)
        for mt in range(TPC):
            pss = [
                ps_pool.tile([P, 512], F32, tag="mm", name=f"pso{c}_{mt}_{h}")
                for h in range(NH)
            ]
            for k in range(K_TILES):
                lhsT = xch[c][:, k * CHUNK_T + mt * P : k * CHUNK_T + (mt + 1) * P]
                for h in range(NH):
                    nc.tensor.matmul(
                        pss[h][:],
                        lhsT=lhsT,
                        rhs=weff[k][:, ts(h, 512)],
                        start=(k == 0),
                        stop=(k == K_TILES - 1),
                    )
            o_sb = o_pool.tile([P, D_OUT], F32, tag="o")
            last = c == N_CHUNKS - 1 and mt == TPC - 1
            if last:
                # tail: scale straight out of psum per half and overlap the
                # two half out-DMAs with the second DVE multiply
                for h in range(NH):
                    nc.vector.tensor_mul(
                        o_sb[:, ts(h, 512)], pss[h][:], s_rep[:, ts(h, 512)]
                    )
                    nc.sync.dma_start(
                        out[ts(c * TPC + mt, P), ts(h, 512)], o_sb[:, ts(h, 512)]
                    )
            else:
                for h in range(NH):
                    # plain drain (no s dependency) so psum slots recycle
                    # immediately; the scale is applied in SBUF afterwards
                    nc.scalar.copy(o_sb[:, ts(h, 512)], pss[h][:])
                # s-scale halves on DVE and gpsimd (both SBUF-only) so
                # neither engine becomes the drain-pipeline pacer
                nc.vector.tensor_mul(
                    o_sb[:, ts(0, 512)], o_sb[:, ts(0, 512)], s_rep[:, ts(0, 512)]
                )
                nc.gpsimd.tensor_mul(
                    o_sb[:, ts(1, 512)], o_sb[:, ts(1, 512)], s_rep[:, ts(1, 512)]
                )
                nc.sync.dma_start(out[ts(c * TPC + mt, P), :], o_sb[:])


def build_nc() -> "bass.Bass":
    nc = bacc.Bacc(
        "TRN2",
        target_bir_lowering=False,
        debug=False,
        num_devices=N_CORES,
    )
    xp = nc.dram_tensor("xp", [M_CORE // CHUNK_T * P, K_TILES * CHUNK_T], BF16,
                        kind="ExternalInput").ap()
    wT = nc.dram_tensor("wT", [D_IN, D_OUT], BF16, kind="ExternalInput").ap()
    abp = nc.dram_tensor("abp", [R, 2 * D_IN], BF16, kind="ExternalInput").ap()
    mag = nc.dram_tensor("mag", [1, D_OUT], F32, kind="ExternalInput").ap()
    out = nc.dram_tensor("out", [M_CORE, D_OUT], F32, kind="ExternalOutput").ap()

    with tile.TileContext(nc) as tc, ExitStack() as ctx:
        _kernel_body(ctx, tc, xp, wT, abp, mag, out)
    nc.compile()
    return nc


_NC_CACHE: list = []


def get_nc() -> "bass.Bass":
    if not _NC_CACHE:
        _NC_CACHE.append(build_nc())
    return _NC_CACHE[0]


def make_in_maps(x, weight, a_w, b_w, magnitude):
    xf = x.reshape(M_TOT, D_IN).astype(NPBF16)
    # per-core d-major chunk layout: xp[c*128+p, k*512+t] = x_core[c*512+t, k*128+p]
    xcs = xf.reshape(N_CORES, N_CHUNKS, CHUNK_T, K_TILES, P)
    xcs = np.ascontiguousarray(xcs.transpose(0, 1, 4, 3, 2))
    xcs = xcs.reshape(N_CORES, N_CHUNKS * P, K_TILES * CHUNK_T)
    wTb = np.ascontiguousarray(weight.astype(np.float32, copy=False).T).astype(NPBF16)
    abp = np.empty((R, 2 * D_IN), NPBF16)
    abp[:, :D_IN] = (SCALING * a_w).astype(NPBF16)
    abp[:, D_IN:] = b_w.astype(np.float32, copy=False).T.astype(NPBF16)
    mag = np.ascontiguousarray(magnitude.astype(np.float32, copy=False))
    return [
        {
            "xp": xcs[i],
            "wT": wTb,
            "abp": abp,
            "mag": mag,
        }
        for i in range(N_CORES)
    ]


def kernel(x, weight, a_w, b_w, magnitude):
    nc = get_nc()
    in_maps = make_in_maps(x, weight, a_w, b_w, magnitude)
    trace = os.environ.get("KERNEL_TRACE", "0") == "1"
    res = run_bass_kernel_spmd(nc, in_maps, list(range(N_CORES)), trace=trace)
    if trace:
        kernel.last_result = res
    outs = [res.results[i]["out"] for i in range(N_CORES)]
    return np.concatenate(outs, axis=0).reshape(B, S, D_OUT)


# revision 18
# speedup vs baseline: 1.0204x; 1.0204x over previous
"""Trainium2 Bass kernel for nn_LoRALinear (DoRA-style LoRA linear).

Reference math (per problem):
    base = x @ W^T
    lora = sc * (x @ A^T) @ B^T          (sc = 2.0)
    w_eff = W + sc * (B @ A)
    s = magnitude / ||w_eff||_row         (row norm over in_dim)
    out = base + (s - 1) * base + s * lora = s * (base + lora)
        = x @ (s[:, None] * w_eff)^T

So the whole op collapses to one dense matmul with a derived weight.

Strategy: data-parallel shard x over batch*seq across 8 cores; every core
redundantly derives w_eff (bf16) + row norms + scale s on device from the
small replicated weights, then computes its x-shard's matmul in bf16
(fp32 accumulate) on the PE array.

The host pre-stages layout only: x is transposed to d-major tiles and
rounded to bf16 (as are W^T / A / B^T), so the device spends zero PE
cycles transposing and streams the matmul at 1 column/cycle. All
arithmetic (w_eff derivation, norms, rsqrt, scaling, the big matmul)
runs on device.

Schedule notes (from trace analysis):
  - dummy warm-up matmuls ramp the PE p-state while the first DMAs land
  - x chunks stream on the gpsimd DGE queue so out-DMA semaphore waits
    on the sync queue can never delay an x prefetch
  - w_eff adds split across DVE (h=0) and gpsimd (h=1) so the setup
    pace matches the PE; BAT psum has its own 2-buf tag so the main
    6-buf rotation never couples to setup latencies
  - the first two token tiles interleave with the 8-step w_eff loop
    k-by-k; norm matmuls run after them, so s (needed only by output
    scaling) is off the PE critical path
  - PSUM drains are plain ACT copies (no s dependency) so psum recycles
    immediately; the s scale is applied by DVE in SBUF afterwards
"""

import os
import numpy as np
from contextlib import ExitStack

import ml_dtypes

import concourse.bass as bass
import concourse.mybir as mybir
import concourse.tile as tile
from concourse import bacc
from concourse.bass import ts
from concourse.bass_utils import run_bass_kernel_spmd

N_CORES = 8
B, S, D_IN, D_OUT, R = 4, 8192, 1024, 1024, 16
SCALING = 32.0 / 16.0
M_TOT = B * S                 # 32768 tokens
M_CORE = M_TOT // N_CORES     # 4096 tokens per core
P = 128
CHUNK_T = 512                 # tokens per DMA chunk
N_CHUNKS = M_CORE // CHUNK_T  # 8
TPC = CHUNK_T // P            # 4 t-tiles per chunk
K_TILES = D_IN // P           # 8
NH = D_OUT // 512             # 2 n-halves of 512
N_WARM = 13                   # PE p-state warm-up matmuls
F32 = mybir.dt.float32
BF16 = mybir.dt.bfloat16
NPBF16 = ml_dtypes.bfloat16


def _kernel_body(ctx: ExitStack, tc: "tile.TileContext", xp, wT, abp, mag, out):
    nc = tc.nc
    const_pool = ctx.enter_context(tc.tile_pool(name="const", bufs=1))
    wt_pool = ctx.enter_context(tc.tile_pool(name="wt", bufs=8))
    w_pool = ctx.enter_context(tc.tile_pool(name="w", bufs=1))
    sq_pool = ctx.enter_context(tc.tile_pool(name="sq", bufs=8))
    x_pool = ctx.enter_context(tc.tile_pool(name="x", bufs=4))
    o_pool = ctx.enter_context(tc.tile_pool(name="o", bufs=8))
    ps_pool = ctx.enter_context(tc.tile_pool(name="ps", bufs=4, space="PSUM"))
    ps_bat = ctx.enter_context(tc.tile_pool(name="ps_bat", bufs=2, space="PSUM"))
    dram_pool = ctx.enter_context(tc.tile_pool(name="dram", bufs=1, space="DRAM"))

    # ---- PE warm-up: dependency-free matmuls ramp the clock while the
    # first DMAs are still in flight ----
    warm_in = const_pool.tile([P, 512], BF16)
    nc.vector.memset(warm_in[:], 0.0)

    def warm_mm(i):
        wps = ps_pool.tile([P, 512], F32, tag="mm", name=f"warm{i}")
        nc.tensor.matmul(
            wps[:], lhsT=warm_in[:, :P], rhs=warm_in[:], start=True, stop=True
        )

    for i in range(N_WARM):
        warm_mm(i)

    # ---- x chunk prefetch on the gpsimd DGE queue; 4 chunks buffered.
    # Chunk 0 loads in two halves so the first k-tiles land sooner. ----
    xch = []
    HK = K_TILES * CHUNK_T // 2

    def issue_x(c):
        t_ = x_pool.tile([P, K_TILES * CHUNK_T], BF16, tag="x", name=f"x{c}")
        if c == 0:
            nc.gpsimd.dma_start(out=t_[:, :HK], in_=xp[ts(c, P), :HK])
            nc.gpsimd.dma_start(out=t_[:, HK:], in_=xp[ts(c, P), HK:])
        else:
            nc.gpsimd.dma_start(out=t_[:], in_=xp[ts(c, P), :])
        xch.append(t_)

    for c in range(4):
        issue_x(c)

    # ---- constants / small inputs ----
    ab_sb = const_pool.tile([R, 2 * D_IN], BF16)
    nc.sync.dma_start(ab_sb[:], abp[:, :])
    a2v = ab_sb[:, :D_IN]
    bTv = ab_sb[:, D_IN:]
    mag_sb = const_pool.tile([1, D_OUT], F32)
    nc.scalar.dma_start(mag_sb[:], mag[:, :])
    ones_f = const_pool.tile([P, 1], F32)
    nc.vector.memset(ones_f[:], 1.0)
    ones = const_pool.tile([P, 1], BF16)
    nc.vector.tensor_copy(ones[:], ones_f[:])
    # prewarm the Ln/Exp ACT tables during the idle head so the s-chain
    # doesn't pay the 1.3us table loads mid-kernel
    tbl = const_pool.tile([1, 1], F32)
    nc.scalar.activation(tbl[:], ones_f[:1, :1], mybir.ActivationFunctionType.Ln)
    nc.scalar.activation(
        tbl[:], ones_f[:1, :1], mybir.ActivationFunctionType.Exp, bias=0.0, scale=-0.5
    )

    # ---- w_eff^T derivation. The DVE add chain (1 full-width add per k,
    # 2-deep BAT psum) is the pace-setter; the PE runs filler warm-ups
    # alongside so its clock stays at peak without coupling to the adds.
    weff = []
    sqs = []
    for k in range(K_TILES):
        wt = wt_pool.tile([P, D_OUT], BF16, tag="wt", name=f"wt{k}")
        nc.sync.dma_start(wt[:], wT[ts(k, P), :])
        for i in range(3):
            warm_mm(100 + 3 * k + i)
        weff_k = w_pool.tile([P, D_OUT], BF16, tag=f"weff{k}", name=f"weff{k}")
        bat = ps_bat.tile([P, D_OUT], F32, tag="bat", name=f"bat{k}")
        for h in range(NH):
            nc.tensor.matmul(
                bat[:, ts(h, 512)],
                lhsT=a2v[:, ts(k, P)],
                rhs=bTv[:, ts(h, 512)],
                start=True,
                stop=True,
            )
        # one full-width fp32 add on DVE, rounded to bf16 on write
        nc.vector.tensor_add(weff_k[:], wt[:], bat[:])
        sqt = sq_pool.tile([P, D_OUT], BF16, tag="sq", name=f"sq{k}")
        nc.scalar.square(sqt[:], weff_k[:])
        sqs.append(sqt)
        weff.append(weff_k)

    # ---- chunk-0 token tiles: matmul + ACT drain now; the s scale and
    # the store are deferred until s_rep exists (DVE s-chain ops must
    # precede any drain multiply in DVE program order) ----
    def t_tile_mms(c, mt):
        pss = [
            ps_pool.tile([P, 512], F32, tag="mm", name=f"pso{c}_{mt}_{h}")
            for h in range(NH)
        ]
        for k in range(K_TILES):
            lhsT = xch[c][:, k * CHUNK_T + mt * P : k * CHUNK_T + (mt + 1) * P]
            for h in range(NH):
                nc.tensor.matmul(
                    pss[h][:],
                    lhsT=lhsT,
                    rhs=weff[k][:, ts(h, 512)],
                    start=(k == 0),
                    stop=(k == K_TILES - 1),
                )
        return pss

    def scale_store(m, o_sb):
        # whole-tile s multiply, alternating engines (both SBUF-only) so
        # neither becomes the drain pacer and they never share a tile
        eng = nc.vector if m % 2 == 0 else nc.gpsimd
        eng.tensor_mul(o_sb[:], o_sb[:], s_rep[:])
        nc.sync.dma_start(out[ts(m, P), :], o_sb[:])

    deferred = []
    for mt in range(TPC):
        pss = t_tile_mms(0, mt)
        o_sb = o_pool.tile([P, D_OUT], F32, tag="o", name=f"o0_{mt}")
        for h in range(NH):
            nc.scalar.copy(o_sb[:, ts(h, 512)], pss[h][:])
        deferred.append((mt, o_sb))

    # ---- row-norm^2 via ones-matmul over squared tiles (PE work placed
    # after chunk 0 so sq_7 is long since ready) ----
    norm2_ps = [
        ps_bat.tile([1, 512], F32, tag="bat", name=f"norm2_{h}") for h in range(NH)
    ]
    for k in range(K_TILES):
        for h in range(NH):
            nc.tensor.matmul(
                norm2_ps[h][:],
                lhsT=ones[:],
                rhs=sqs[k][:, ts(h, 512)],
                start=(k == 0),
                stop=(k == K_TILES - 1),
            )

    # ---- s = mag / sqrt(norm2); broadcast to all partitions ----
    norm2_sb = const_pool.tile([1, D_OUT], F32)
    for h in range(NH):
        nc.scalar.copy(norm2_sb[:, ts(h, 512)], norm2_ps[h][:])
    # rsqrt(n) = exp(-0.5 * ln(n)); LUT error is well inside tolerance
    lnn = const_pool.tile([1, D_OUT], F32)
    nc.scalar.activation(lnn[:], norm2_sb[:], mybir.ActivationFunctionType.Ln)
    y = const_pool.tile([1, D_OUT], F32)
    nc.scalar.activation(
        y[:], lnn[:], mybir.ActivationFunctionType.Exp, bias=0.0, scale=-0.5
    )
    s1 = const_pool.tile([1, D_OUT], F32)
    nc.vector.tensor_mul(s1[:], mag_sb[:], y[:])
    # broadcast s to all 128 partitions via a DRAM round trip with a
    # stride-0 partition read
    s_dram = dram_pool.tile([1, D_OUT], F32)
    nc.sync.dma_start(s_dram[:], s1[:])
    sd = s_dram[:]
    s_bcast_ap = bass.AP(tensor=sd.tensor, offset=sd.offset, ap=[[0, P], *sd.ap])
    s_rep = const_pool.tile([P, D_OUT], F32)
    nc.sync.dma_start(out=s_rep[:], in_=s_bcast_ap)

    # deferred chunk-0 scale + store
    for mt, o_sb in deferred:
        scale_store(mt, o_sb)

    # ---- main loop over 512-token chunks ----
    # xp rows c*128+p hold x^T data: xp[c*128+p, k*512+t] = x[c*512+t, k*128+p]
    for c in range(1, N_CHUNKS):
        if c + 3 < N_CHUNKS:
            issue_x(c + 3)
        for mt in range(TPC):
            pss = t_tile_mms(c, mt)
            m = c * TPC + mt
            o_sb = o_pool.tile([P, D_OUT], F32, tag="o")
            if c == N_CHUNKS - 1 and mt == TPC - 1:
                # tail: scale straight out of psum per half and overlap the
                # two half out-DMAs with the second DVE multiply
                for h in range(NH):
                    nc.vector.tensor_mul(
                        o_sb[:, ts(h, 512)], pss[h][:], s_rep[:, ts(h, 512)]
                    )
                    nc.sync.dma_start(
                        out[ts(m, P), ts(h, 512)], o_sb[:, ts(h, 512)]
                    )
            else:
                for h in range(NH):
                    # plain drain (no s dependency) so psum slots recycle
                    # immediately; the scale is applied in SBUF afterwards
                    nc.scalar.copy(o_sb[:, ts(h, 512)], pss[h][:])
                scale_store(m, o_sb)


def build_nc() -> "bass.Bass":
    nc = bacc.Bacc(
        "TRN2",
        target_bir_lowering=False,
        debug=False,
        num_devices=N_CORES,
    )
    xp = nc.dram_tensor("xp", [M_CORE // CHUNK_T * P, K_TILES * CHUNK_T], BF16,
                        kind="ExternalInput").ap()
    wT = nc.dram_tensor("wT", [D_IN, D_OUT], BF16, kind="ExternalInput").ap()
    abp = nc.dram_tensor("abp", [R, 2 * D_IN], BF16, kind="ExternalInput").ap()
    mag = nc.dram_tensor("mag", [1, D_OUT], F32, kind="ExternalInput").ap()
    out = nc.dram_tensor("out", [M_CORE, D_OUT], F32, kind="ExternalOutput").ap()

    with tile.TileContext(nc) as tc, ExitStack() as ctx:
        _kernel_body(ctx, tc, xp, wT, abp, mag, out)
    nc.compile()
    return nc


_NC_CACHE: list = []


def get_nc() -> "bass.Bass":
    if not _NC_CACHE:
        _NC_CACHE.append(build_nc())
    return _NC_CACHE[0]


def make_in_maps(x, weight, a_w, b_w, magnitude):
    xf = x.reshape(M_TOT, D_IN).astype(NPBF16)
    # per-core d-major chunk layout: xp[c*128+p, k*512+t] = x_core[c*512+t, k*128+p]
    xcs = xf.reshape(N_CORES, N_CHUNKS, CHUNK_T, K_TILES, P)
    xcs = np.ascontiguousarray(xcs.transpose(0, 1, 4, 3, 2))
    xcs = xcs.reshape(N_CORES, N_CHUNKS * P, K_TILES * CHUNK_T)
    wTb = np.ascontiguousarray(weight.astype(np.float32, copy=False).T).astype(NPBF16)
    abp = np.empty((R, 2 * D_IN), NPBF16)
    abp[:, :D_IN] = (SCALING * a_w).astype(NPBF16)
    abp[:, D_IN:] = b_w.astype(np.float32, copy=False).T.astype(NPBF16)
    mag = np.ascontiguousarray(magnitude.astype(np.float32, copy=False))
    return [
        {
            "xp": xcs[i],
            "wT": wTb,
            "abp": abp,
            "mag": mag,
        }
        for i in range(N_CORES)
    ]


def kernel(x, weight, a_w, b_w, magnitude):
    nc = get_nc()
    in_maps = make_in_maps(x, weight, a_w, b_w, magnitude)
    trace = os.environ.get("KERNEL_TRACE", "0") == "1"
    res = run_bass_kernel_spmd(nc, in_maps, list(range(N_CORES)), trace=trace)
    if trace:
        kernel.last_result = res
    outs = [res.results[i]["out"] for i in range(N_CORES)]
    return np.concatenate(outs, axis=0).reshape(B, S, D_OUT)


# revision 19
# speedup vs baseline: 1.0375x; 1.0167x over previous
"""Trainium2 Bass kernel for nn_LoRALinear (DoRA-style LoRA linear).

Reference math (per problem):
    base = x @ W^T
    lora = sc * (x @ A^T) @ B^T          (sc = 2.0)
    w_eff = W + sc * (B @ A)
    s = magnitude / ||w_eff||_row         (row norm over in_dim)
    out = base + (s - 1) * base + s * lora = s * (base + lora)
        = x @ (s[:, None] * w_eff)^T

So the whole op collapses to one dense matmul with a derived weight.

Strategy: data-parallel shard x over batch*seq across 8 cores; every core
redundantly derives w_eff (bf16) + row norms + scale s on device from the
small replicated weights, then computes its x-shard's matmul in bf16
(fp32 accumulate) on the PE array.

The host pre-stages layout only: x is transposed to d-major tiles and
rounded to bf16 (as are W^T / A / B^T), so the device spends zero PE
cycles transposing and streams the matmul at 1 column/cycle. All
arithmetic (w_eff derivation, norms, rsqrt, scaling, the big matmul)
runs on device.

Schedule notes (from trace analysis):
  - dummy warm-up matmuls ramp the PE p-state while the first DMAs land
  - x chunks stream on the gpsimd DGE queue so out-DMA semaphore waits
    on the sync queue can never delay an x prefetch
  - w_eff adds split across DVE (h=0) and gpsimd (h=1) so the setup
    pace matches the PE; BAT psum has its own 2-buf tag so the main
    6-buf rotation never couples to setup latencies
  - the first two token tiles interleave with the 8-step w_eff loop
    k-by-k; norm matmuls run after them, so s (needed only by output
    scaling) is off the PE critical path
  - PSUM drains are plain ACT copies (no s dependency) so psum recycles
    immediately; the s scale is applied by DVE in SBUF afterwards
"""

import os
import numpy as np
from contextlib import ExitStack

import ml_dtypes

import concourse.bass as bass
import concourse.mybir as mybir
import concourse.tile as tile
from concourse import bacc
from concourse.bass import ts
from concourse.bass_utils import run_bass_kernel_spmd

N_CORES = 8
B, S, D_IN, D_OUT, R = 4, 8192, 1024, 1024, 16
SCALING = 32.0 / 16.0
M_TOT = B * S                 # 32768 tokens
M_CORE = M_TOT // N_CORES     # 4096 tokens per core
P = 128
CHUNK_T = 512                 # tokens per DMA chunk
N_CHUNKS = M_CORE // CHUNK_T  # 8
TPC = CHUNK_T // P            # 4 t-tiles per chunk
K_TILES = D_IN // P           # 8
NH = D_OUT // 512             # 2 n-halves of 512
N_WARM = 13                   # PE p-state warm-up matmuls
F32 = mybir.dt.float32
BF16 = mybir.dt.bfloat16
NPBF16 = ml_dtypes.bfloat16


def _kernel_body(ctx: ExitStack, tc: "tile.TileContext", xp, wT, abp, mag, out):
    nc = tc.nc
    const_pool = ctx.enter_context(tc.tile_pool(name="const", bufs=1))
    wt_pool = ctx.enter_context(tc.tile_pool(name="wt", bufs=8))
    w_pool = ctx.enter_context(tc.tile_pool(name="w", bufs=1))
    sq_pool = ctx.enter_context(tc.tile_pool(name="sq", bufs=8))
    x_pool = ctx.enter_context(tc.tile_pool(name="x", bufs=4))
    o_pool = ctx.enter_context(tc.tile_pool(name="o", bufs=8))
    ps_pool = ctx.enter_context(tc.tile_pool(name="ps", bufs=5, space="PSUM"))
    ps_bat = ctx.enter_context(tc.tile_pool(name="ps_bat", bufs=2, space="PSUM"))
    ps_warm = ctx.enter_context(tc.tile_pool(name="ps_warm", bufs=1, space="PSUM"))
    dram_pool = ctx.enter_context(tc.tile_pool(name="dram", bufs=1, space="DRAM"))

    # ---- PE warm-up / filler matmuls: dependency-free, all into one
    # dedicated psum tile, so they keep the PE clock at peak whenever the
    # real work stream has a bubble without touching any rotation ----
    warm_in = const_pool.tile([P, 512], BF16)
    nc.vector.memset(warm_in[:], 0.0)
    warm_ps = ps_warm.tile([P, 512], F32, tag="warm", name="warm_ps")

    def warm_mm():
        nc.tensor.matmul(
            warm_ps[:], lhsT=warm_in[:, :P], rhs=warm_in[:], start=True, stop=True
        )

    for i in range(N_WARM):
        warm_mm()

    # ---- x chunk staging on the gpsimd DGE queue. Only chunk 0 loads
    # upfront (in halves) so the weight stream isn't starved on the wire;
    # later chunks issue as earlier compute retires. ----
    xch = []
    HK = K_TILES * CHUNK_T // 2

    def issue_x(c):
        t_ = x_pool.tile([P, K_TILES * CHUNK_T], BF16, tag="x", name=f"x{c}")
        if c == 0:
            nc.gpsimd.dma_start(out=t_[:, :HK], in_=xp[ts(c, P), :HK])
            nc.gpsimd.dma_start(out=t_[:, HK:], in_=xp[ts(c, P), HK:])
        else:
            nc.gpsimd.dma_start(out=t_[:], in_=xp[ts(c, P), :])
        xch.append(t_)

    issue_x(0)

    # ---- constants / small inputs ----
    ab_sb = const_pool.tile([R, 2 * D_IN], BF16)
    nc.sync.dma_start(ab_sb[:], abp[:, :])
    a2v = ab_sb[:, :D_IN]
    bTv = ab_sb[:, D_IN:]
    mag_sb = const_pool.tile([1, D_OUT], F32)
    nc.scalar.dma_start(mag_sb[:], mag[:, :])
    ones_f = const_pool.tile([P, 1], F32)
    nc.vector.memset(ones_f[:], 1.0)
    ones = const_pool.tile([P, 1], BF16)
    nc.vector.tensor_copy(ones[:], ones_f[:])
    # prewarm the Ln/Exp ACT tables during the idle head so the s-chain
    # doesn't pay the 1.3us table loads mid-kernel
    tbl = const_pool.tile([1, 1], F32)
    nc.scalar.activation(tbl[:], ones_f[:1, :1], mybir.ActivationFunctionType.Ln)
    nc.scalar.activation(
        tbl[:], ones_f[:1, :1], mybir.ActivationFunctionType.Exp, bias=0.0, scale=-0.5
    )

    # ---- w_eff^T derivation with the first two token tiles interleaved
    # k-by-k; filler warm-ups absorb the DVE add-chain latency ----
    t01_ps = [
        [
            ps_pool.tile([P, 512], F32, tag="mm", name=f"pst{t}_{h}")
            for h in range(NH)
        ]
        for t in range(2)
    ]
    weff = []
    sqs = []
    for k in range(K_TILES):
        wt = wt_pool.tile([P, D_OUT], BF16, tag="wt", name=f"wt{k}")
        nc.sync.dma_start(wt[:], wT[ts(k, P), :])
        warm_mm()
        warm_mm()
        weff_k = w_pool.tile([P, D_OUT], BF16, tag=f"weff{k}", name=f"weff{k}")
        for h in range(NH):
            bat = ps_bat.tile([P, 512], F32, tag="bat", name=f"bat{k}_{h}")
            nc.tensor.matmul(
                bat[:],
                lhsT=a2v[:, ts(k, P)],
                rhs=bTv[:, ts(h, 512)],
                start=True,
                stop=True,
            )
            # fp32 add on DVE, rounded to bf16 on write
            nc.vector.tensor_add(weff_k[:, ts(h, 512)], wt[:, ts(h, 512)], bat[:])
        sqt = sq_pool.tile([P, D_OUT], BF16, tag="sq", name=f"sq{k}")
        nc.scalar.square(sqt[:], weff_k[:])
        sqs.append(sqt)
        for t in range(2):
            for h in range(NH):
                nc.tensor.matmul(
                    t01_ps[t][h][:],
                    lhsT=xch[0][:, t * P + k * CHUNK_T : (t + 1) * P + k * CHUNK_T],
                    rhs=weff_k[:, ts(h, 512)],
                    start=(k == 0),
                    stop=(k == K_TILES - 1),
                )
        weff.append(weff_k)

    issue_x(1)

    # t0/t1 psum drains (plain ACT copies; the s scale comes later, after
    # the s-chain DVE ops are queued)
    deferred = []
    for t in range(2):
        o_sb = o_pool.tile([P, D_OUT], F32, tag="o", name=f"o01_{t}")
        for h in range(NH):
            nc.scalar.copy(o_sb[:, ts(h, 512)], t01_ps[t][h][:])
        deferred.append((t, o_sb))

    # ---- row-norm^2 via ones-matmul over squared tiles ----
    norm2_ps = [
        ps_bat.tile([1, 512], F32, tag="bat", name=f"norm2_{h}") for h in range(NH)
    ]
    for k in range(K_TILES):
        for h in range(NH):
            nc.tensor.matmul(
                norm2_ps[h][:],
                lhsT=ones[:],
                rhs=sqs[k][:, ts(h, 512)],
                start=(k == 0),
                stop=(k == K_TILES - 1),
            )

    # ---- s = mag / sqrt(norm2); broadcast to all partitions ----
    norm2_sb = const_pool.tile([1, D_OUT], F32)
    for h in range(NH):
        nc.scalar.copy(norm2_sb[:, ts(h, 512)], norm2_ps[h][:])
    # rsqrt(n) = exp(-0.5 * ln(n)); LUT error is well inside tolerance
    lnn = const_pool.tile([1, D_OUT], F32)
    nc.scalar.activation(lnn[:], norm2_sb[:], mybir.ActivationFunctionType.Ln)
    y = const_pool.tile([1, D_OUT], F32)
    nc.scalar.activation(
        y[:], lnn[:], mybir.ActivationFunctionType.Exp, bias=0.0, scale=-0.5
    )
    s1 = const_pool.tile([1, D_OUT], F32)
    nc.vector.tensor_mul(s1[:], mag_sb[:], y[:])
    # broadcast s to all 128 partitions via a DRAM round trip with a
    # stride-0 partition read
    s_dram = dram_pool.tile([1, D_OUT], F32)
    nc.sync.dma_start(s_dram[:], s1[:])
    sd = s_dram[:]
    s_bcast_ap = bass.AP(tensor=sd.tensor, offset=sd.offset, ap=[[0, P], *sd.ap])
    s_rep = const_pool.tile([P, D_OUT], F32)
    nc.sync.dma_start(out=s_rep[:], in_=s_bcast_ap)

    def scale_store(m, o_sb):
        # whole-tile s multiply, alternating engines (both SBUF-only) so
        # neither becomes the drain pacer and they never share a tile
        eng = nc.vector if m % 2 == 0 else nc.gpsimd
        eng.tensor_mul(o_sb[:], o_sb[:], s_rep[:])
        nc.sync.dma_start(out[ts(m, P), :], o_sb[:])

    issue_x(2)
    issue_x(3)

    # deferred t0/t1 scale + store
    for m, o_sb in deferred:
        scale_store(m, o_sb)

    # ---- main loop over 512-token chunks ----
    # xp rows c*128+p hold x^T data: xp[c*128+p, k*512+t] = x[c*512+t, k*128+p]
    for c in range(N_CHUNKS):
        if 2 <= c and c + 2 < N_CHUNKS:
            issue_x(c + 2)
        for mt in range(2 if c == 0 else 0, TPC):
            pss = [
                ps_pool.tile([P, 512], F32, tag="mm", name=f"pso{c}_{mt}_{h}")
                for h in range(NH)
            ]
            for k in range(K_TILES):
                lhsT = xch[c][:, k * CHUNK_T + mt * P : k * CHUNK_T + (mt + 1) * P]
                for h in range(NH):
                    nc.tensor.matmul(
                        pss[h][:],
                        lhsT=lhsT,
                        rhs=weff[k][:, ts(h, 512)],
                        start=(k == 0),
                        stop=(k == K_TILES - 1),
                    )
            m = c * TPC + mt
            o_sb = o_pool.tile([P, D_OUT], F32, tag="o")
            if c == N_CHUNKS - 1 and mt == TPC - 1:
                # tail: scale straight out of psum per half and overlap the
                # two half out-DMAs with the second DVE multiply
                for h in range(NH):
                    nc.vector.tensor_mul(
                        o_sb[:, ts(h, 512)], pss[h][:], s_rep[:, ts(h, 512)]
                    )
                    nc.sync.dma_start(
                        out[ts(m, P), ts(h, 512)], o_sb[:, ts(h, 512)]
                    )
            else:
                for h in range(NH):
                    # plain drain (no s dependency) so psum slots recycle
                    # immediately; the scale is applied in SBUF afterwards
                    nc.scalar.copy(o_sb[:, ts(h, 512)], pss[h][:])
                scale_store(m, o_sb)


def build_nc() -> "bass.Bass":
    nc = bacc.Bacc(
        "TRN2",
        target_bir_lowering=False,
        debug=False,
        num_devices=N_CORES,
    )
    xp = nc.dram_tensor("xp", [M_CORE // CHUNK_T * P, K_TILES * CHUNK_T], BF16,
                        kind="ExternalInput").ap()
    wT = nc.dram_tensor("wT", [D_IN, D_OUT], BF16, kind="ExternalInput").ap()
    abp = nc.dram_tensor("abp", [R, 2 * D_IN], BF16, kind="ExternalInput").ap()
    mag = nc.dram_tensor("mag", [1, D_OUT], F32, kind="ExternalInput").ap()
    out = nc.dram_tensor("out", [M_CORE, D_OUT], F32, kind="ExternalOutput").ap()

    with tile.TileContext(nc) as tc, ExitStack() as ctx:
        _kernel_body(ctx, tc, xp, wT, abp, mag, out)
    nc.compile()
    return nc


_NC_CACHE: list = []


def get_nc() -> "bass.Bass":
    if not _NC_CACHE:
        _NC_CACHE.append(build_nc())
    return _NC_CACHE[0]


def make_in_maps(x, weight, a_w, b_w, magnitude):
    xf = x.reshape(M_TOT, D_IN).astype(NPBF16)
    # per-core d-major chunk layout: xp[c*128+p, k*512+t] = x_core[c*512+t, k*128+p]
    xcs = xf.reshape(N_CORES, N_CHUNKS, CHUNK_T, K_TILES, P)
    xcs = np.ascontiguousarray(xcs.transpose(0, 1, 4, 3, 2))
    xcs = xcs.reshape(N_CORES, N_CHUNKS * P, K_TILES * CHUNK_T)
    wTb = np.ascontiguousarray(weight.astype(np.float32, copy=False).T).astype(NPBF16)
    abp = np.empty((R, 2 * D_IN), NPBF16)
    abp[:, :D_IN] = (SCALING * a_w).astype(NPBF16)
    abp[:, D_IN:] = b_w.astype(np.float32, copy=False).T.astype(NPBF16)
    mag = np.ascontiguousarray(magnitude.astype(np.float32, copy=False))
    return [
        {
            "xp": xcs[i],
            "wT": wTb,
            "abp": abp,
            "mag": mag,
        }
        for i in range(N_CORES)
    ]


def kernel(x, weight, a_w, b_w, magnitude):
    nc = get_nc()
    in_maps = make_in_maps(x, weight, a_w, b_w, magnitude)
    trace = os.environ.get("KERNEL_TRACE", "0") == "1"
    res = run_bass_kernel_spmd(nc, in_maps, list(range(N_CORES)), trace=trace)
    if trace:
        kernel.last_result = res
    outs = [res.results[i]["out"] for i in range(N_CORES)]
    return np.concatenate(outs, axis=0).reshape(B, S, D_OUT)


# revision 20
# speedup vs baseline: 1.0574x; 1.0192x over previous
"""Trainium2 Bass kernel for nn_LoRALinear (DoRA-style LoRA linear).

Reference math (per problem):
    base = x @ W^T
    lora = sc * (x @ A^T) @ B^T          (sc = 2.0)
    w_eff = W + sc * (B @ A)
    s = magnitude / ||w_eff||_row         (row norm over in_dim)
    out = base + (s - 1) * base + s * lora = s * (base + lora)
        = x @ (s[:, None] * w_eff)^T

So the whole op collapses to one dense matmul with a derived weight.

Strategy: data-parallel shard x over batch*seq across 8 cores; every core
redundantly derives w_eff (bf16) + row norms + scale s on device from the
small replicated weights, then computes its x-shard's matmul in bf16
(fp32 accumulate) on the PE array.

The host pre-stages layout only: x is transposed to d-major tiles and
rounded to bf16 (as are W^T / A / B^T), so the device spends zero PE
cycles transposing and streams the matmul at 1 column/cycle. All
arithmetic (w_eff derivation, norms, rsqrt, scaling, the big matmul)
runs on device.

Schedule notes (from trace analysis):
  - dummy warm-up matmuls ramp the PE p-state while the first DMAs land
  - x chunks stream on the gpsimd DGE queue so out-DMA semaphore waits
    on the sync queue can never delay an x prefetch
  - w_eff adds split across DVE (h=0) and gpsimd (h=1) so the setup
    pace matches the PE; BAT psum has its own 2-buf tag so the main
    6-buf rotation never couples to setup latencies
  - the first two token tiles interleave with the 8-step w_eff loop
    k-by-k; norm matmuls run after them, so s (needed only by output
    scaling) is off the PE critical path
  - PSUM drains are plain ACT copies (no s dependency) so psum recycles
    immediately; the s scale is applied by DVE in SBUF afterwards
"""

import os
import numpy as np
from contextlib import ExitStack

import ml_dtypes

import concourse.bass as bass
import concourse.mybir as mybir
import concourse.tile as tile
from concourse import bacc
from concourse.bass import ts
from concourse.bass_utils import run_bass_kernel_spmd
from concourse.masks import make_identity

N_CORES = 8
B, S, D_IN, D_OUT, R = 4, 8192, 1024, 1024, 16
SCALING = 32.0 / 16.0
M_TOT = B * S                 # 32768 tokens
M_CORE = M_TOT // N_CORES     # 4096 tokens per core
P = 128
CHUNK_T = 512                 # tokens per DMA chunk
N_CHUNKS = M_CORE // CHUNK_T  # 8
TPC = CHUNK_T // P            # 4 t-tiles per chunk
K_TILES = D_IN // P           # 8
NH = D_OUT // 512             # 2 n-halves of 512
N_WARM = 9                   # PE p-state warm-up matmuls
F32 = mybir.dt.float32
BF16 = mybir.dt.bfloat16
NPBF16 = ml_dtypes.bfloat16


def _kernel_body(ctx: ExitStack, tc: "tile.TileContext", xp, wT, abp, mag, out):
    nc = tc.nc
    const_pool = ctx.enter_context(tc.tile_pool(name="const", bufs=1))
    wt_pool = ctx.enter_context(tc.tile_pool(name="wt", bufs=8))
    w_pool = ctx.enter_context(tc.tile_pool(name="w", bufs=1))
    sq_pool = ctx.enter_context(tc.tile_pool(name="sq", bufs=8))
    x_pool = ctx.enter_context(tc.tile_pool(name="x", bufs=4))
    o_pool = ctx.enter_context(tc.tile_pool(name="o", bufs=8))
    ps_pool = ctx.enter_context(tc.tile_pool(name="ps", bufs=4, space="PSUM"))
    ps_bat = ctx.enter_context(tc.tile_pool(name="ps_bat", bufs=3, space="PSUM"))
    ps_warm = ctx.enter_context(tc.tile_pool(name="ps_warm", bufs=1, space="PSUM"))
    dram_pool = ctx.enter_context(tc.tile_pool(name="dram", bufs=1, space="DRAM"))

    # ---- PE warm-up / filler matmuls: dependency-free, all into one
    # dedicated psum tile, so they keep the PE clock at peak whenever the
    # real work stream has a bubble without touching any rotation ----
    warm_in = const_pool.tile([P, 512], BF16)
    nc.vector.memset(warm_in[:], 0.0)
    warm_ps = ps_warm.tile([P, 512], F32, tag="warm", name="warm_ps")

    def warm_mm():
        nc.tensor.matmul(
            warm_ps[:], lhsT=warm_in[:, :P], rhs=warm_in[:], start=True, stop=True
        )

    for i in range(N_WARM):
        warm_mm()

    # ---- x chunk staging on the gpsimd DGE queue. Only chunk 0 loads
    # upfront (in halves) so the weight stream isn't starved on the wire;
    # later chunks issue as earlier compute retires. ----
    xch = []
    HK = K_TILES * CHUNK_T // 2

    def issue_x(c):
        t_ = x_pool.tile([P, K_TILES * CHUNK_T], BF16, tag="x", name=f"x{c}")
        if c == 0:
            nc.gpsimd.dma_start(out=t_[:, :HK], in_=xp[ts(c, P), :HK])
            nc.gpsimd.dma_start(out=t_[:, HK:], in_=xp[ts(c, P), HK:])
        else:
            nc.gpsimd.dma_start(out=t_[:], in_=xp[ts(c, P), :])
        xch.append(t_)

    issue_x(0)

    # ---- constants / small inputs ----
    ab_sb = const_pool.tile([R, 2 * D_IN], BF16)
    nc.sync.dma_start(ab_sb[:], abp[:, :])
    a2v = ab_sb[:, :D_IN]
    bTv = ab_sb[:, D_IN:]
    mag_sb = const_pool.tile([1, D_OUT], F32)
    nc.scalar.dma_start(mag_sb[:], mag[:, :])
    ones_f = const_pool.tile([P, 1], F32)
    nc.vector.memset(ones_f[:], 1.0)
    ones = const_pool.tile([P, 1], BF16)
    nc.vector.tensor_copy(ones[:], ones_f[:])
    # prewarm the Ln/Exp ACT tables during the idle head so the s-chain
    # doesn't pay the 1.3us table loads mid-kernel
    tbl = const_pool.tile([1, 1], F32)
    nc.scalar.activation(tbl[:], ones_f[:1, :1], mybir.ActivationFunctionType.Ln)
    nc.scalar.activation(
        tbl[:], ones_f[:1, :1], mybir.ActivationFunctionType.Exp, bias=0.0, scale=-0.5
    )
    ident = const_pool.tile([P, P], BF16)
    make_identity(nc, ident[:])

    # ---- w_eff^T derivation with the first two token tiles interleaved
    # k-by-k; filler warm-ups absorb the DVE add-chain latency ----
    t01_ps = [
        [
            ps_pool.tile([P, 512], F32, tag="mm", name=f"pst{t}_{h}")
            for h in range(NH)
        ]
        for t in range(2)
    ]
    weff = []
    sqs = []
    for k in range(K_TILES):
        wt = wt_pool.tile([P, D_OUT], BF16, tag="wt", name=f"wt{k}")
        nc.sync.dma_start(wt[:], wT[ts(k, P), :])
        weff_k = w_pool.tile([P, D_OUT], BF16, tag=f"weff{k}", name=f"weff{k}")
        for h in range(NH):
            bat = ps_bat.tile([P, 512], F32, tag="bat", name=f"bat{k}_{h}")
            # psum = (2BA)^T slice, then += W^T slice via identity matmul:
            # the whole w_eff add runs on the PE, no cross-engine chain
            nc.tensor.matmul(
                bat[:],
                lhsT=a2v[:, ts(k, P)],
                rhs=bTv[:, ts(h, 512)],
                start=True,
                stop=False,
            )
            nc.tensor.matmul(
                bat[:],
                lhsT=ident[:],
                rhs=wt[:, ts(h, 512)],
                start=False,
                stop=True,
            )
            # rounding drain psum -> bf16 on ACT
            nc.scalar.copy(weff_k[:, ts(h, 512)], bat[:])
        sqt = sq_pool.tile([P, D_OUT], BF16, tag="sq", name=f"sq{k}")
        nc.vector.tensor_mul(sqt[:], weff_k[:], weff_k[:])
        sqs.append(sqt)
        for t in range(2):
            for h in range(NH):
                nc.tensor.matmul(
                    t01_ps[t][h][:],
                    lhsT=xch[0][:, t * P + k * CHUNK_T : (t + 1) * P + k * CHUNK_T],
                    rhs=weff_k[:, ts(h, 512)],
                    start=(k == 0),
                    stop=(k == K_TILES - 1),
                )
        weff.append(weff_k)

    issue_x(1)

    # t0/t1 psum drains (plain ACT copies; the s scale comes later, after
    # the s-chain DVE ops are queued)
    deferred = []
    for t in range(2):
        o_sb = o_pool.tile([P, D_OUT], F32, tag="o", name=f"o01_{t}")
        for h in range(NH):
            nc.scalar.copy(o_sb[:, ts(h, 512)], t01_ps[t][h][:])
        deferred.append((t, o_sb))

    # ---- row-norm^2 via ones-matmul over squared tiles ----
    norm2_ps = [
        ps_bat.tile([1, 512], F32, tag="bat", name=f"norm2_{h}") for h in range(NH)
    ]
    for k in range(K_TILES):
        for h in range(NH):
            nc.tensor.matmul(
                norm2_ps[h][:],
                lhsT=ones[:],
                rhs=sqs[k][:, ts(h, 512)],
                start=(k == 0),
                stop=(k == K_TILES - 1),
            )

    # ---- s = mag / sqrt(norm2); broadcast to all partitions ----
    norm2_sb = const_pool.tile([1, D_OUT], F32)
    for h in range(NH):
        nc.scalar.copy(norm2_sb[:, ts(h, 512)], norm2_ps[h][:])
    # rsqrt(n) = exp(-0.5 * ln(n)); LUT error is well inside tolerance
    lnn = const_pool.tile([1, D_OUT], F32)
    nc.scalar.activation(lnn[:], norm2_sb[:], mybir.ActivationFunctionType.Ln)
    y = const_pool.tile([1, D_OUT], F32)
    nc.scalar.activation(
        y[:], lnn[:], mybir.ActivationFunctionType.Exp, bias=0.0, scale=-0.5
    )
    s1 = const_pool.tile([1, D_OUT], F32)
    nc.vector.tensor_mul(s1[:], mag_sb[:], y[:])
    # broadcast s to all 128 partitions via a DRAM round trip with a
    # stride-0 partition read
    s_dram = dram_pool.tile([1, D_OUT], F32)
    nc.sync.dma_start(s_dram[:], s1[:])
    sd = s_dram[:]
    s_bcast_ap = bass.AP(tensor=sd.tensor, offset=sd.offset, ap=[[0, P], *sd.ap])
    s_rep = const_pool.tile([P, D_OUT], F32)
    nc.sync.dma_start(out=s_rep[:], in_=s_bcast_ap)

    def scale_store(m, o_sb):
        # whole-tile s multiply, alternating engines (both SBUF-only) so
        # neither becomes the drain pacer and they never share a tile
        eng = nc.vector if m % 2 == 0 else nc.gpsimd
        eng.tensor_mul(o_sb[:], o_sb[:], s_rep[:])
        nc.sync.dma_start(out[ts(m, P), :], o_sb[:])

    issue_x(2)
    issue_x(3)

    # deferred t0/t1 scale + store
    for m, o_sb in deferred:
        scale_store(m, o_sb)

    # ---- main loop over 512-token chunks ----
    # xp rows c*128+p hold x^T data: xp[c*128+p, k*512+t] = x[c*512+t, k*128+p]
    for c in range(N_CHUNKS):
        if 2 <= c and c + 2 < N_CHUNKS:
            issue_x(c + 2)
        for mt in range(2 if c == 0 else 0, TPC):
            pss = [
                ps_pool.tile([P, 512], F32, tag="mm", name=f"pso{c}_{mt}_{h}")
                for h in range(NH)
            ]
            for k in range(K_TILES):
                lhsT = xch[c][:, k * CHUNK_T + mt * P : k * CHUNK_T + (mt + 1) * P]
                for h in range(NH):
                    nc.tensor.matmul(
                        pss[h][:],
                        lhsT=lhsT,
                        rhs=weff[k][:, ts(h, 512)],
                        start=(k == 0),
                        stop=(k == K_TILES - 1),
                    )
            m = c * TPC + mt
            o_sb = o_pool.tile([P, D_OUT], F32, tag="o")
            if c == N_CHUNKS - 1 and mt == TPC - 1:
                # tail: scale straight out of psum per half and overlap the
                # two half out-DMAs with the second DVE multiply
                for h in range(NH):
                    nc.vector.tensor_mul(
                        o_sb[:, ts(h, 512)], pss[h][:], s_rep[:, ts(h, 512)]
                    )
                    nc.sync.dma_start(
                        out[ts(m, P), ts(h, 512)], o_sb[:, ts(h, 512)]
                    )
            else:
                for h in range(NH):
                    # plain drain (no s dependency) so psum slots recycle
                    # immediately; the scale is applied in SBUF afterwards
                    nc.scalar.copy(o_sb[:, ts(h, 512)], pss[h][:])
                scale_store(m, o_sb)


def build_nc() -> "bass.Bass":
    nc = bacc.Bacc(
        "TRN2",
        target_bir_lowering=False,
        debug=False,
        num_devices=N_CORES,
    )
    xp = nc.dram_tensor("xp", [M_CORE // CHUNK_T * P, K_TILES * CHUNK_T], BF16,
                        kind="ExternalInput").ap()
    wT = nc.dram_tensor("wT", [D_IN, D_OUT], BF16, kind="ExternalInput").ap()
    abp = nc.dram_tensor("abp", [R, 2 * D_IN], BF16, kind="ExternalInput").ap()
    mag = nc.dram_tensor("mag", [1, D_OUT], F32, kind="ExternalInput").ap()
    out = nc.dram_tensor("out", [M_CORE, D_OUT], F32, kind="ExternalOutput").ap()

    with tile.TileContext(nc) as tc, ExitStack() as ctx:
        _kernel_body(ctx, tc, xp, wT, abp, mag, out)
    nc.compile()
    return nc


_NC_CACHE: list = []


def get_nc() -> "bass.Bass":
    if not _NC_CACHE:
        _NC_CACHE.append(build_nc())
    return _NC_CACHE[0]


def make_in_maps(x, weight, a_w, b_w, magnitude):
    xf = x.reshape(M_TOT, D_IN).astype(NPBF16)
    # per-core d-major chunk layout: xp[c*128+p, k*512+t] = x_core[c*512+t, k*128+p]
    xcs = xf.reshape(N_CORES, N_CHUNKS, CHUNK_T, K_TILES, P)
    xcs = np.ascontiguousarray(xcs.transpose(0, 1, 4, 3, 2))
    xcs = xcs.reshape(N_CORES, N_CHUNKS * P, K_TILES * CHUNK_T)
    wTb = np.ascontiguousarray(weight.astype(np.float32, copy=False).T).astype(NPBF16)
    abp = np.empty((R, 2 * D_IN), NPBF16)
    abp[:, :D_IN] = (SCALING * a_w).astype(NPBF16)
    abp[:, D_IN:] = b_w.astype(np.float32, copy=False).T.astype(NPBF16)
    mag = np.ascontiguousarray(magnitude.astype(np.float32, copy=False))
    return [
        {
            "xp": xcs[i],
            "wT": wTb,
            "abp": abp,
            "mag": mag,
        }
        for i in range(N_CORES)
    ]


def kernel(x, weight, a_w, b_w, magnitude):
    nc = get_nc()
    in_maps = make_in_maps(x, weight, a_w, b_w, magnitude)
    trace = os.environ.get("KERNEL_TRACE", "0") == "1"
    res = run_bass_kernel_spmd(nc, in_maps, list(range(N_CORES)), trace=trace)
    if trace:
        kernel.last_result = res
    outs = [res.results[i]["out"] for i in range(N_CORES)]
    return np.concatenate(outs, axis=0).reshape(B, S, D_OUT)
